# revision 10
# baseline (speedup 1.0000x reference)
"""Causal self-attention kernel for 8 TRN2 NeuronCores.

Sharding: core = b*4 + g  (b = batch 0..1, g = head-group 0..3, 4 heads each).
Each core computes, for its batch b and its 4 heads:
  qkv projection -> per-head causal attention (softmax without max-subtraction,
  scores are bounded ~N(0,1)) -> partial output projection over its 256
  attn columns.  Host sums the 4 per-batch partials and adds the bias.

On-device layout (per core):
  xT     [E=1024, S=2048]  host-pretransposed x[b].T  (contraction dim on partitions)
  wqkvT  [E, F=768]        host-built [Wq_g; Wk_g; Wv_g].T
  woutT  [256, E]          host-built w_out[:, 256g:256g+256].T
  mask   [128, 4, 512]     causal mask tiles for the diagonal superblocks
  out    [S, E]            partial output (pre-bias)

All matmuls run as float32r (full-rate fp32 on the PE array).
"""

import os

import numpy as np

_B, _S, _E = 2, 2048, 1024
_H, _D = 16, 64
_F = 768  # per-core qkv rows: 4 heads * 3 * 64
_P = 128

# stash of the last profiled exec time (ns), for test harnesses
LAST_EXEC_TIME_NS = None

_PROGRAM_CACHE = {}


def _build_program(S=_S):
    import concourse.bacc as bacc
    import concourse.mybir as mybir
    import concourse.tile as tile

    f32 = mybir.dt.float32
    f32r = mybir.dt.float32r
    cdt = f32r  # compute dtype for all matmul operands
    Exp = mybir.ActivationFunctionType.Exp

    P = _P
    E, F = _E, _F
    NCH = E // P          # 8 contraction chunks for the projections
    NSB = S // 512        # s-blocks of 512
    NST = S // 128        # s-tiles of 128
    NIB = S // 512        # i-blocks (attention query blocks)

    nc = bacc.Bacc("TRN2", target_bir_lowering=False, debug=False)

    xT = nc.declare_dram_parameter("xT", [E, S], cdt, isOutput=False)
    wqkvT = nc.declare_dram_parameter("wqkvT", [E, F], cdt, isOutput=False)
    woutT = nc.declare_dram_parameter("woutT", [256, E], cdt, isOutput=False)
    maskd = nc.declare_dram_parameter("mask", [P, 4, 512], cdt, isOutput=False)
    outd = nc.declare_dram_parameter("out", [S, E], f32, isOutput=True)

    x3 = xT[:].rearrange("(ko p) s -> p ko s", p=P)      # [128, 8, S]
    w3 = wqkvT[:].rearrange("(ko p) f -> p ko f", p=P)   # [128, 8, 768]
    wo3 = woutT[:].rearrange("(c p) e -> p c e", p=P)    # [128, 2, 1024]

    with tile.TileContext(nc) as tc:
        with (
            tc.tile_pool(name="consts", bufs=1) as consts,
            tc.tile_pool(name="xpool", bufs=2) as xpool,
            tc.tile_pool(name="qkpool", bufs=1) as qkpool,
            tc.tile_pool(name="vpool", bufs=1) as vpool,
            tc.tile_pool(name="atpool", bufs=1) as atpool,
            tc.tile_pool(name="probs", bufs=3) as probs,
            tc.tile_pool(name="small", bufs=4) as small,
            tc.tile_pool(name="outpool", bufs=3) as outpool,
            tc.tile_pool(name="psum", bufs=2, space="PSUM") as psum,
        ):
            # ---- constants ----
            w_sb = consts.tile([P, NCH, F], cdt)
            nc.sync.dma_start(w_sb[:], w3[:])
            wo_sb = consts.tile([P, 2, E], cdt)
            nc.sync.dma_start(wo_sb[:], wo3[:])
            mask_sb = consts.tile([P, 4, 512], cdt)
            nc.sync.dma_start(mask_sb[:], maskd[:])

            # persistent activations
            qkT = qkpool.tile([P, 4, S], cdt)      # ko 0-1: qT heads {0,1},{2,3}; 2-3: kT
            v_sb = vpool.tile([P, NST, 4 * 65], cdt)
            v4 = v_sb.rearrange("p t (h e) -> p t h e", h=4)  # [128, NST, 4, 65]
            attnT = atpool.tile([P, 2, S], cdt)    # [64*(h%2)+d, h//2, i] normalized attn.T

            # ones column of v_aug (row-sum trick for softmax denominators);
            # mask[:, 3, 511] is all-ones, broadcast-DMA it into the gaps
            nc.sync.dma_start(
                v_sb.rearrange("p t e -> p (t e)")[:, 64::65],
                maskd[:, 3, 511:512].to_broadcast((P, NST * 4)),
            )

            # ---- phase B: qkv projections ----
            for sbk in range(NSB):
                s0 = 512 * sbk
                xt = xpool.tile([P, NCH, 512], cdt, tag="xt", name=f"xt{sbk}")
                nc.sync.dma_start(xt[:], x3[:, :, s0:s0 + 512])
                # qT / kT : out rows = W columns (f), weights stationary
                for ft in range(4):
                    qkps = psum.tile([P, 512], f32, tag="acc", bufs=3,
                                     name=f"qkps{sbk}_{ft}")
                    for ch in range(NCH):
                        nc.tensor.matmul(
                            qkps[:],
                            lhsT=w_sb[:, ch, 128 * ft:128 * (ft + 1)],
                            rhs=xt[:, ch, :],
                            start=(ch == 0), stop=(ch == NCH - 1),
                        )
                    nc.vector.tensor_copy(qkT[:, ft, s0:s0 + 512], qkps[:])
                # v : out rows = s, x stationary
                for st in range(4):
                    gst = 4 * sbk + st
                    vps = psum.tile([P, 256], f32, tag="acc", bufs=3,
                                    name=f"vps{sbk}_{st}")
                    for ch in range(NCH):
                        nc.tensor.matmul(
                            vps[:],
                            lhsT=xt[:, ch, 128 * st:128 * (st + 1)],
                            rhs=w_sb[:, ch, 512:768],
                            start=(ch == 0), stop=(ch == NCH - 1),
                        )
                    nc.vector.tensor_copy(
                        v4[:, gst, :, 0:64],
                        vps.rearrange("p (h e) -> p h e", h=4),
                    )

            # ---- phase C: per-head causal attention ----
            for h in range(4):
                po = 64 * (h % 2)
                koq = h // 2
                kok = 2 + h // 2
                for ib in range(NIB):
                    i0 = 512 * ib
                    atps = psum.tile([65, 512], f32, tag="acc", bufs=3,
                                     name=f"atps{h}_{ib}")
                    nsb = 2 * (ib + 1)  # superblocks of 2 j-tiles (causal)
                    for sb2 in range(nsb):
                        scps = psum.tile([P, 2, 512], f32, tag="sc", bufs=2,
                                         name=f"scps{h}_{ib}_{sb2}")
                        for jj in range(2):
                            jt = 2 * sb2 + jj
                            nc.tensor.matmul(
                                scps[:, jj],
                                lhsT=qkT[po:po + 64, kok,
                                         128 * jt:128 * (jt + 1)],
                                rhs=qkT[po:po + 64, koq, i0:i0 + 512],
                                start=True, stop=True, skip_group_check=True,
                            )
                        pb = probs.tile([P, 2, 512], cdt, tag="pb", name=f"pb{h}_{ib}_{sb2}")
                        # probs = exp(scores / sqrt(D)); no max-subtraction needed
                        nc.scalar.activation(pb[:], scps[:], Exp, scale=0.125)
                        if sb2 >= 2 * ib:  # diagonal superblock: causal mask
                            moff = 2 * (sb2 - 2 * ib)
                            nc.vector.tensor_mul(
                                pb[:], pb[:], mask_sb[:, moff:moff + 2, :]
                            )
                        for jj in range(2):
                            jt = 2 * sb2 + jj
                            nc.tensor.matmul(
                                atps[:],
                                lhsT=v4[:, jt, h, :],
                                rhs=pb[:, jj],
                                start=(sb2 == 0 and jj == 0),
                                stop=(sb2 == nsb - 1 and jj == 1),
                                skip_group_check=True,
                            )
                    # normalize: attnT = attn_unnorm / rowsum (row 64 of atps)
                    rec = small.tile([1, 512], f32, tag="rec", name=f"rec{h}_{ib}")
                    nc.vector.reciprocal(rec[:], atps[64:65, :])
                    bc = small.tile([64, 512], f32, tag="bc", name=f"bc{h}_{ib}")
                    nc.gpsimd.partition_broadcast(bc[:], rec[:])
                    nc.vector.tensor_mul(
                        attnT[po:po + 64, koq, i0:i0 + 512], atps[0:64, :], bc[:]
                    )

            # ---- phase D: output projection (partial; host sums over cores) ----
            for it in range(NST):
                ot = outpool.tile([P, E], f32, tag="ot", name=f"ot{it}")
                for ec in range(2):
                    ops = psum.tile([P, 512], f32, tag="acc", bufs=3,
                                    name=f"ops{it}_{ec}")
                    for c in range(2):
                        nc.tensor.matmul(
                            ops[:],
                            lhsT=attnT[:, c, 128 * it:128 * (it + 1)],
                            rhs=wo_sb[:, c, 512 * ec:512 * (ec + 1)],
                            start=(c == 0), stop=(c == 1), skip_group_check=True,
                        )
                    nc.vector.tensor_copy(ot[:, 512 * ec:512 * (ec + 1)], ops[:])
                nc.sync.dma_start(outd[128 * it:128 * (it + 1), :], ot[:])

    nc.compile()
    return nc


def _get_program(S=_S):
    if S not in _PROGRAM_CACHE:
        _PROGRAM_CACHE[S] = _build_program(S)
    return _PROGRAM_CACHE[S]


def _make_mask():
    pp = np.arange(_P)[:, None, None]
    tt = np.arange(4)[None, :, None]
    cc = np.arange(512)[None, None, :]
    return ((128 * tt + pp) <= cc).astype(np.float32)


def _round_fp32r(a):
    """Round float32 to the fp32r grid (11-bit mantissa, low 12 bits zero)."""
    bits = np.ascontiguousarray(a, np.float32).view(np.uint32)
    r = ((bits.astype(np.uint64) + 0x800) & 0xFFFFF000).astype(np.uint32)
    return r.view(np.float32)


def make_in_maps(x, w_qkv, w_out):
    x = np.ascontiguousarray(np.asarray(x, np.float32))
    w_qkv = np.ascontiguousarray(np.asarray(w_qkv, np.float32))
    w_out = np.ascontiguousarray(np.asarray(w_out, np.float32))
    E = _E
    mask = _make_mask()
    xTs = [_round_fp32r(x[b].T) for b in range(_B)]
    wqs, wos = [], []
    for g in range(4):
        W = np.concatenate(
            [
                w_qkv[256 * g:256 * (g + 1)],
                w_qkv[E + 256 * g:E + 256 * (g + 1)],
                w_qkv[2 * E + 256 * g:2 * E + 256 * (g + 1)],
            ],
            axis=0,
        )  # [768, E]
        wqs.append(_round_fp32r(W.T))                        # [E, 768]
        wos.append(_round_fp32r(w_out[:, 256 * g:256 * (g + 1)].T))  # [256, E]
    in_maps = []
    for core in range(8):
        b, g = core // 4, core % 4
        in_maps.append(
            {"xT": xTs[b], "wqkvT": wqs[g], "woutT": wos[g], "mask": mask}
        )
    return in_maps


LAST_TRACE_DIR = None


def _enable_jax_compile_cache():
    try:
        import jax

        jax.config.update("jax_compilation_cache_dir", "/tmp/jax_cache")
        jax.config.update("jax_persistent_cache_min_compile_time_secs", 0.0)
        jax.config.update("jax_persistent_cache_min_entry_size_bytes", -1)
    except Exception:
        pass


def kernel(x, w_qkv, w_out, b_out):
    global LAST_EXEC_TIME_NS, LAST_TRACE_DIR
    from concourse.bass_utils import run_bass_kernel_spmd

    _enable_jax_compile_cache()
    b_out = np.asarray(b_out, np.float32)
    in_maps = make_in_maps(x, w_qkv, w_out)
    nc = _get_program()
    trace = bool(int(os.environ.get("BASS_PROFILE", "0")))
    tmpdir = None
    if trace:
        import tempfile

        tmpdir = tempfile.mkdtemp(prefix="bass_trace_")
        LAST_TRACE_DIR = tmpdir
    res = run_bass_kernel_spmd(
        nc, in_maps, core_ids=list(range(8)), trace=trace, tmpdir=tmpdir
    )
    LAST_EXEC_TIME_NS = res.exec_time_ns
    out = np.zeros((_B, _S, _E), np.float32)
    for core in range(8):
        out[core // 4] += res.results[core]["out"]
    out += b_out[None, None, :]
    return out


# revision 14
# speedup vs baseline: 1.0550x; 1.0550x over previous
"""Causal self-attention kernel for 8 TRN2 NeuronCores.

Sharding: core = b*4 + g  (b = batch 0..1, g = head-group 0..3, 4 heads each).
Each core computes, for its batch b and its 4 heads:
  qkv projection -> per-head causal attention (softmax without max-subtraction,
  scores are bounded ~N(0,1)) -> partial output projection over its 256
  attn columns.  Host sums the 4 per-batch partials and adds the bias.

On-device layout (per core):
  xT     [E=1024, S=2048]  host-pretransposed x[b].T  (contraction dim on partitions)
  wqkvT  [E, F=768]        host-built [Wq_g; Wk_g; Wv_g].T
  woutT  [256, E]          host-built w_out[:, 256g:256g+256].T
  mask   [128, 4, 512]     causal mask tiles for the diagonal superblocks
  out    [S, E]            partial output (pre-bias)

All matmuls run as float32r (full-rate fp32 on the PE array).
"""

import os

import numpy as np

_B, _S, _E = 2, 2048, 1024
_H, _D = 16, 64
_F = 768  # per-core qkv rows: 4 heads * 3 * 64
_P = 128

# stash of the last profiled exec time (ns), for test harnesses
LAST_EXEC_TIME_NS = None

_PROGRAM_CACHE = {}


def _build_program(S=_S):
    import concourse.bacc as bacc
    import concourse.mybir as mybir
    import concourse.tile as tile

    f32 = mybir.dt.float32
    f32r = mybir.dt.float32r
    cdt = f32r  # compute dtype for all matmul operands
    Exp = mybir.ActivationFunctionType.Exp

    P = _P
    E, F = _E, _F
    NCH = E // P          # 8 contraction chunks for the projections
    NSB = S // 512        # s-blocks of 512
    NST = S // 128        # s-tiles of 128
    NIB = S // 512        # i-blocks (attention query blocks)

    nc = bacc.Bacc("TRN2", target_bir_lowering=False, debug=False)

    xT = nc.declare_dram_parameter("xT", [E, S], cdt, isOutput=False)
    wqkvT = nc.declare_dram_parameter("wqkvT", [E, F], cdt, isOutput=False)
    woutT = nc.declare_dram_parameter("woutT", [256, E], cdt, isOutput=False)
    maskd = nc.declare_dram_parameter("mask", [P, 4, 512], cdt, isOutput=False)
    outd = nc.declare_dram_parameter("out", [S, E], f32, isOutput=True)

    x3 = xT[:].rearrange("(ko p) s -> p ko s", p=P)      # [128, 8, S]
    w3 = wqkvT[:].rearrange("(ko p) f -> p ko f", p=P)   # [128, 8, 768]
    wo3 = woutT[:].rearrange("(c p) e -> p c e", p=P)    # [128, 2, 1024]

    with tile.TileContext(nc) as tc:
        with (
            tc.tile_pool(name="consts", bufs=1) as consts,
            tc.tile_pool(name="xpool", bufs=2) as xpool,
            tc.tile_pool(name="qkpool", bufs=1) as qkpool,
            tc.tile_pool(name="vpool", bufs=1) as vpool,
            tc.tile_pool(name="atpool", bufs=1) as atpool,
            tc.tile_pool(name="probs", bufs=3) as probs,
            tc.tile_pool(name="small", bufs=4) as small,
            tc.tile_pool(name="outpool", bufs=3) as outpool,
            tc.tile_pool(name="psum", bufs=2, space="PSUM") as psum,
        ):
            # ---- constants ----
            w_sb = consts.tile([P, NCH, F], cdt)
            for ch in range(NCH):
                nc.sync.dma_start(w_sb[:, ch], w3[:, ch])
            wo_sb = consts.tile([P, 2, E], cdt)
            mask_sb = consts.tile([P, 4, 512], cdt)

            # persistent activations
            qkT = qkpool.tile([P, 4, S], cdt)      # ko 0-1: qT heads {0,1},{2,3}; 2-3: kT
            v_sb = vpool.tile([P, NST, 4 * 65], cdt)
            v4 = v_sb.rearrange("p t (h e) -> p t h e", h=4)  # [128, NST, 4, 65]
            attnT = atpool.tile([P, 2, S], cdt)    # [64*(h%2)+d, h//2, i] normalized attn.T

            # ones column of v_aug (row-sum trick for softmax denominators);
            # mask[:, 3, 511] is all-ones, broadcast-DMA it into the gaps
            nc.sync.dma_start(
                v_sb.rearrange("p t e -> p (t e)")[:, 64::65],
                maskd[:, 3, 511:512].to_broadcast((P, NST * 4)),
            )

            # ---- phase B: qkv projections ----
            for sbk in range(NSB):
                s0 = 512 * sbk
                xt = xpool.tile([P, NCH, 512], cdt, tag="xt", name=f"xt{sbk}")
                for ch in range(NCH):
                    nc.sync.dma_start(xt[:, ch], x3[:, ch, s0:s0 + 512])
                # qT / kT : out rows = W columns (f), weights stationary
                for ft in range(4):
                    qkps = psum.tile([P, 512], f32, tag="acc", bufs=4,
                                     name=f"qkps{sbk}_{ft}")
                    for ch in range(NCH):
                        nc.tensor.matmul(
                            qkps[:],
                            lhsT=w_sb[:, ch, 128 * ft:128 * (ft + 1)],
                            rhs=xt[:, ch, :],
                            start=(ch == 0), stop=(ch == NCH - 1),
                        )
                    nc.vector.tensor_copy(qkT[:, ft, s0:s0 + 512], qkps[:])
                # v : out rows = s, x stationary
                for st in range(4):
                    gst = 4 * sbk + st
                    vps = psum.tile([P, 256], f32, tag="acc", bufs=4,
                                    name=f"vps{sbk}_{st}")
                    for ch in range(NCH):
                        nc.tensor.matmul(
                            vps[:],
                            lhsT=xt[:, ch, 128 * st:128 * (st + 1)],
                            rhs=w_sb[:, ch, 512:768],
                            start=(ch == 0), stop=(ch == NCH - 1),
                        )
                    nc.vector.tensor_copy(
                        v4[:, gst, :, 0:64],
                        vps.rearrange("p (h e) -> p h e", h=4),
                    )

            nc.sync.dma_start(mask_sb[:], maskd[:])
            nc.sync.dma_start(wo_sb[:], wo3[:])

            # ---- phase C+D: causal attention (ib outer) with fused out-proj ----
            Copy = mybir.ActivationFunctionType.Copy
            for ib in range(NIB):
                i0 = 512 * ib
                for h in range(4):
                    po = 64 * (h % 2)
                    koq = h // 2
                    kok = 2 + h // 2
                    atps = psum.tile([65, 512], f32, tag="acc", bufs=4,
                                     name=f"atps{h}_{ib}")
                    nsb = 2 * (ib + 1)  # superblocks of 2 j-tiles (causal)
                    for sb2 in range(nsb):
                        diag = sb2 >= 2 * ib
                        scps = psum.tile([P, 2, 512], f32, tag="sc", bufs=2,
                                         name=f"scps{h}_{ib}_{sb2}")
                        pb = probs.tile([P, 2, 512], cdt, tag="pb",
                                        name=f"pb{h}_{ib}_{sb2}")
                        # columns < 128*t of diagonal j-tile t are fully
                        # masked; skip them (keep f32r moving dim >= 256)
                        c0s = []
                        for jj in range(2):
                            jt = 2 * sb2 + jj
                            t = jt - 4 * ib
                            c0 = min(128 * t, 256) if (diag and t > 0) else 0
                            c0s.append(c0)
                            nc.tensor.matmul(
                                scps[:, jj, c0:],
                                lhsT=qkT[po:po + 64, kok,
                                         128 * jt:128 * (jt + 1)],
                                rhs=qkT[po:po + 64, koq, i0 + c0:i0 + 512],
                                start=True, stop=True, skip_group_check=True,
                            )
                        # probs = exp(scores / sqrt(D)); no max-subtraction
                        if not diag:
                            nc.scalar.activation(pb[:], scps[:], Exp, scale=0.125)
                        for jj in range(2):
                            jt = 2 * sb2 + jj
                            t = jt - 4 * ib
                            c0 = c0s[jj]
                            if diag:
                                nc.scalar.activation(pb[:, jj, c0:],
                                                     scps[:, jj, c0:],
                                                     Exp, scale=0.125)
                                m1 = min(128 * t + 128, 512)
                                nc.vector.tensor_mul(
                                    pb[:, jj, c0:m1], pb[:, jj, c0:m1],
                                    mask_sb[:, t, c0:m1],
                                )
                            nc.tensor.matmul(
                                atps[:, c0:],
                                lhsT=v4[:, jt, h, :],
                                rhs=pb[:, jj, c0:],
                                start=(sb2 == 0 and jj == 0),
                                stop=(sb2 == nsb - 1 and jj == 1),
                                skip_group_check=True,
                            )
                    # normalize: attnT = attn_unnorm / rowsum (row 64 of atps)
                    rs = small.tile([1, 512], f32, tag="rec", name=f"rs{h}_{ib}")
                    nc.scalar.activation(rs[:], atps[64:65, :], Copy)
                    bc = small.tile([64, 512], f32, tag="bc", name=f"bc{h}_{ib}")
                    nc.gpsimd.partition_broadcast(bc[:], rs[:])
                    rec = small.tile([64, 512], f32, tag="rec64", name=f"rc{h}_{ib}")
                    nc.vector.reciprocal(rec[:], bc[:])
                    nc.vector.tensor_mul(
                        attnT[po:po + 64, koq, i0:i0 + 512], atps[0:64, :], rec[:]
                    )
                # out-proj for this i-block (keeps PE fed while ACT runs exp)
                for its in range(4):
                    it = 4 * ib + its
                    ot = outpool.tile([P, E], f32, tag="ot", name=f"ot{it}")
                    for ec in range(2):
                        ops = psum.tile([P, 512], f32, tag="acc", bufs=4,
                                        name=f"ops{it}_{ec}")
                        for c in range(2):
                            nc.tensor.matmul(
                                ops[:],
                                lhsT=attnT[:, c, 128 * it:128 * (it + 1)],
                                rhs=wo_sb[:, c, 512 * ec:512 * (ec + 1)],
                                start=(c == 0), stop=(c == 1),
                                skip_group_check=True,
                            )
                        nc.vector.tensor_copy(ot[:, 512 * ec:512 * (ec + 1)], ops[:])
                    nc.sync.dma_start(outd[128 * it:128 * (it + 1), :], ot[:])

    nc.compile()
    return nc


def _get_program(S=_S):
    if S not in _PROGRAM_CACHE:
        _PROGRAM_CACHE[S] = _build_program(S)
    return _PROGRAM_CACHE[S]


def _make_mask():
    pp = np.arange(_P)[:, None, None]
    tt = np.arange(4)[None, :, None]
    cc = np.arange(512)[None, None, :]
    return ((128 * tt + pp) <= cc).astype(np.float32)


def _round_fp32r(a):
    """Round float32 to the fp32r grid (11-bit mantissa, low 12 bits zero)."""
    bits = np.ascontiguousarray(a, np.float32).view(np.uint32)
    r = ((bits.astype(np.uint64) + 0x800) & 0xFFFFF000).astype(np.uint32)
    return r.view(np.float32)


def make_in_maps(x, w_qkv, w_out):
    x = np.ascontiguousarray(np.asarray(x, np.float32))
    w_qkv = np.ascontiguousarray(np.asarray(w_qkv, np.float32))
    w_out = np.ascontiguousarray(np.asarray(w_out, np.float32))
    E = _E
    mask = _make_mask()
    xTs = [_round_fp32r(x[b].T) for b in range(_B)]
    wqs, wos = [], []
    for g in range(4):
        W = np.concatenate(
            [
                w_qkv[256 * g:256 * (g + 1)],
                w_qkv[E + 256 * g:E + 256 * (g + 1)],
                w_qkv[2 * E + 256 * g:2 * E + 256 * (g + 1)],
            ],
            axis=0,
        )  # [768, E]
        wqs.append(_round_fp32r(W.T))                        # [E, 768]
        wos.append(_round_fp32r(w_out[:, 256 * g:256 * (g + 1)].T))  # [256, E]
    in_maps = []
    for core in range(8):
        b, g = core // 4, core % 4
        in_maps.append(
            {"xT": xTs[b], "wqkvT": wqs[g], "woutT": wos[g], "mask": mask}
        )
    return in_maps


LAST_TRACE_DIR = None


def _enable_jax_compile_cache():
    try:
        import jax

        jax.config.update("jax_compilation_cache_dir", "/tmp/jax_cache")
        jax.config.update("jax_persistent_cache_min_compile_time_secs", 0.0)
        jax.config.update("jax_persistent_cache_min_entry_size_bytes", -1)
    except Exception:
        pass


def kernel(x, w_qkv, w_out, b_out):
    global LAST_EXEC_TIME_NS, LAST_TRACE_DIR
    from concourse.bass_utils import run_bass_kernel_spmd

    _enable_jax_compile_cache()
    b_out = np.asarray(b_out, np.float32)
    in_maps = make_in_maps(x, w_qkv, w_out)
    nc = _get_program()
    trace = bool(int(os.environ.get("BASS_PROFILE", "0")))
    tmpdir = None
    if trace:
        import tempfile

        tmpdir = tempfile.mkdtemp(prefix="bass_trace_")
        LAST_TRACE_DIR = tmpdir
    res = run_bass_kernel_spmd(
        nc, in_maps, core_ids=list(range(8)), trace=trace, tmpdir=tmpdir
    )
    LAST_EXEC_TIME_NS = res.exec_time_ns
    out = np.zeros((_B, _S, _E), np.float32)
    for core in range(8):
        out[core // 4] += res.results[core]["out"]
    out += b_out[None, None, :]
    return out


# revision 16
# speedup vs baseline: 1.2071x; 1.1442x over previous
"""Causal self-attention kernel for 8 TRN2 NeuronCores.

Sharding: core = b*4 + g  (b = batch 0..1, g = head-group 0..3, 4 heads each).
Each core computes, for its batch b and its 4 heads:
  qkv projection -> per-head causal attention (softmax without max-subtraction,
  scores are bounded ~N(0,1)) -> partial output projection over its 256
  attn columns.  Host sums the 4 per-batch partials and adds the bias.

On-device layout (per core):
  xT     [E=1024, S=2048]  host-pretransposed x[b].T  (contraction dim on partitions)
  wqkvT  [E, F=768]        host-built [Wq_g; Wk_g; Wv_g].T
  woutT  [256, E]          host-built w_out[:, 256g:256g+256].T
  mask   [128, 4, 512]     causal mask tiles for the diagonal superblocks
  out    [S, E]            partial output (pre-bias)

All matmuls run as float32r (full-rate fp32 on the PE array).
"""

import os

import numpy as np

_B, _S, _E = 2, 2048, 1024
_H, _D = 16, 64
_F = 768  # per-core qkv rows: 4 heads * 3 * 64
_P = 128

# stash of the last profiled exec time (ns), for test harnesses
LAST_EXEC_TIME_NS = None

_PROGRAM_CACHE = {}


def _build_program(S=_S):
    import concourse.bacc as bacc
    import concourse.mybir as mybir
    import concourse.tile as tile

    f32 = mybir.dt.float32
    f32r = mybir.dt.float32r
    cdt = f32r  # compute dtype for all matmul operands
    Exp = mybir.ActivationFunctionType.Exp

    P = _P
    E, F = _E, _F
    NCH = E // P          # 8 contraction chunks for the projections
    NSB = S // 512        # s-blocks of 512
    NIB = S // 512        # i-blocks (attention query blocks)

    nc = bacc.Bacc("TRN2", target_bir_lowering=False, debug=False)

    xT = nc.declare_dram_parameter("xT", [E, S], cdt, isOutput=False)
    wqkvT = nc.declare_dram_parameter("wqkvT", [E, F], cdt, isOutput=False)
    woutT = nc.declare_dram_parameter("woutT", [256, E], cdt, isOutput=False)
    maskd = nc.declare_dram_parameter("mask", [P, 4, 512], cdt, isOutput=False)
    outd = nc.declare_dram_parameter("out", [S, E], f32, isOutput=True)

    x3 = xT[:].rearrange("(ko p) s -> p ko s", p=P)      # [128, 8, S]
    w3 = wqkvT[:].rearrange("(ko p) f -> p ko f", p=P)   # [128, 8, 768]
    wo3 = woutT[:].rearrange("(c p) e -> p c e", p=P)    # [128, 2, 1024]

    with tile.TileContext(nc) as tc:
        with (
            tc.tile_pool(name="consts", bufs=1) as consts,
            tc.tile_pool(name="xpool", bufs=2) as xpool,
            tc.tile_pool(name="qkpool", bufs=1) as qkpool,
            tc.tile_pool(name="vpool", bufs=1) as vpool,
            tc.tile_pool(name="atpool", bufs=1) as atpool,
            tc.tile_pool(name="probs", bufs=3) as probs,
            tc.tile_pool(name="small", bufs=4) as small,
            tc.tile_pool(name="outpool", bufs=3) as outpool,
            tc.tile_pool(name="psum", bufs=2, space="PSUM") as psum,
        ):
            # ---- constants ----
            w_sb = consts.tile([P, NCH, F], cdt)
            for ch in range(NCH):
                nc.sync.dma_start(w_sb[:, ch], w3[:, ch])
            wo_sb = consts.tile([P, 2, E], cdt)
            mask_sb = consts.tile([P, 4, 512], cdt)

            # per-s-block persistent activations (split tiles so the Tile
            # scheduler can start attention as soon as its block is ready)
            qk_t = [qkpool.tile([P, 4, 512], cdt, name=f"qk{s}") for s in range(NSB)]
            v_t = [vpool.tile([P, 4, 4 * 65], cdt, name=f"v{s}") for s in range(NSB)]
            at_t = [atpool.tile([P, 2, 512], cdt, name=f"at{s}") for s in range(NIB)]
            v4 = [v_t[s].rearrange("p t (h e) -> p t h e", h=4) for s in range(NSB)]

            # ones column of v_aug (row-sum trick for softmax denominators);
            # mask[:, 3, 511] is all-ones, broadcast-DMA it into the gaps
            for s in range(NSB):
                nc.sync.dma_start(
                    v_t[s].rearrange("p t e -> p (t e)")[:, 64::65],
                    maskd[:, 3, 511:512].to_broadcast((P, 4 * 4)),
                )

            def emit_proj(sbk):
                """qkv projection for s-block sbk (PE-dense filler work)."""
                s0 = 512 * sbk
                xt = xpool.tile([P, NCH, 512], cdt, tag="xt", name=f"xt{sbk}")
                for ch in range(NCH):
                    nc.sync.dma_start(xt[:, ch], x3[:, ch, s0:s0 + 512])
                # qT / kT : out rows = W columns (f), weights stationary
                for ft in range(4):
                    qkps = psum.tile([P, 512], f32, tag="acc", bufs=4,
                                     name=f"qkps{sbk}_{ft}")
                    for ch in range(NCH):
                        nc.tensor.matmul(
                            qkps[:],
                            lhsT=w_sb[:, ch, 128 * ft:128 * (ft + 1)],
                            rhs=xt[:, ch, :],
                            start=(ch == 0), stop=(ch == NCH - 1),
                        )
                    nc.vector.tensor_copy(qk_t[sbk][:, ft, :], qkps[:])
                # v : out rows = s, x stationary
                for st in range(4):
                    vps = psum.tile([P, 256], f32, tag="acc", bufs=4,
                                    name=f"vps{sbk}_{st}")
                    for ch in range(NCH):
                        nc.tensor.matmul(
                            vps[:],
                            lhsT=xt[:, ch, 128 * st:128 * (st + 1)],
                            rhs=w_sb[:, ch, 512:768],
                            start=(ch == 0), stop=(ch == NCH - 1),
                        )
                    nc.vector.tensor_copy(
                        v4[sbk][:, st, :, 0:64],
                        vps.rearrange("p (h e) -> p h e", h=4),
                    )

            emit_proj(0)
            nc.sync.dma_start(mask_sb[:], maskd[:])
            nc.sync.dma_start(wo_sb[:], wo3[:])

            # ---- attention (ib outer), with the next s-block projection and
            # ---- this block's out-projection interleaved to keep PE dense
            for ib in range(NIB):
                i0 = 512 * ib
                for h in range(4):
                    po = 64 * (h % 2)
                    koq = h // 2
                    kok = 2 + h // 2
                    atps = psum.tile([65, 512], f32, tag="acc", bufs=4,
                                     name=f"atps{h}_{ib}")
                    nsb = 2 * (ib + 1)  # superblocks of 2 j-tiles (causal)
                    for sb2 in range(nsb):
                        d0 = sb2 == 2 * ib      # diagonal superblock (t=0,1)
                        d1 = sb2 == 2 * ib + 1  # diagonal superblock (t=2,3)
                        # fully-masked columns skipped on the d1 superblock
                        e0 = 256 if d1 else 0
                        scps = psum.tile([P, 2, 512], f32, tag="sc", bufs=2,
                                         name=f"scps{h}_{ib}_{sb2}")
                        pb = probs.tile([P, 2, 512], cdt, tag="pb",
                                        name=f"pb{h}_{ib}_{sb2}")
                        for jj in range(2):
                            jt = 2 * sb2 + jj
                            lt = jt % 4
                            nc.tensor.matmul(
                                scps[:, jj, e0:],
                                lhsT=qk_t[jt // 4][po:po + 64, kok,
                                                  128 * lt:128 * (lt + 1)],
                                rhs=qk_t[ib][po:po + 64, koq, e0:],
                                start=True, stop=True, skip_group_check=True,
                            )
                        # probs = exp(scores / sqrt(D)); no max-subtraction
                        nc.scalar.activation(pb[:, :, e0:], scps[:, :, e0:],
                                             Exp, scale=0.125)
                        for jj in range(2):
                            jt = 2 * sb2 + jj
                            t = jt - 4 * ib
                            if d0 or d1:
                                # causal mask; extended strip covers the
                                # fully-masked + partial columns
                                m0, m1 = (0, 128 * t + 128) if d0 else (256, 512)
                                nc.vector.tensor_mul(
                                    pb[:, jj, m0:m1], pb[:, jj, m0:m1],
                                    mask_sb[:, t, m0:m1],
                                )
                            c0 = min(128 * t, 256) if (d0 or d1) and t > 0 else 0
                            nc.tensor.matmul(
                                atps[:, c0:],
                                lhsT=v4[jt // 4][:, jt % 4, h, :],
                                rhs=pb[:, jj, c0:],
                                start=(sb2 == 0 and jj == 0),
                                stop=(sb2 == nsb - 1 and jj == 1),
                                skip_group_check=True,
                            )
                    # normalize: attnT = attn_unnorm / rowsum (row 64 of atps)
                    rs = small.tile([1, 512], f32, tag="rec", name=f"rs{h}_{ib}")
                    nc.scalar.activation(rs[:], atps[64:65, :],
                                         mybir.ActivationFunctionType.Copy)
                    bc = small.tile([64, 512], f32, tag="bc", name=f"bc{h}_{ib}")
                    nc.gpsimd.partition_broadcast(bc[:], rs[:])
                    rec = small.tile([64, 512], f32, tag="rec64", name=f"rc{h}_{ib}")
                    nc.vector.reciprocal(rec[:], bc[:])
                    nc.vector.tensor_mul(
                        at_t[ib][po:po + 64, koq, :], atps[0:64, :], rec[:]
                    )
                # next s-block projection: PE filler during exp stalls
                if ib + 1 < NSB:
                    emit_proj(ib + 1)
                # out-proj for this i-block
                for its in range(4):
                    it = 4 * ib + its
                    ot = outpool.tile([P, E], f32, tag="ot", name=f"ot{it}")
                    for ec in range(2):
                        ops = psum.tile([P, 512], f32, tag="acc", bufs=4,
                                        name=f"ops{it}_{ec}")
                        for c in range(2):
                            nc.tensor.matmul(
                                ops[:],
                                lhsT=at_t[ib][:, c, 128 * its:128 * (its + 1)],
                                rhs=wo_sb[:, c, 512 * ec:512 * (ec + 1)],
                                start=(c == 0), stop=(c == 1),
                                skip_group_check=True,
                            )
                        nc.vector.tensor_copy(ot[:, 512 * ec:512 * (ec + 1)], ops[:])
                    nc.sync.dma_start(outd[128 * it:128 * (it + 1), :], ot[:])

    nc.compile()
    return nc


def _get_program(S=_S):
    if S not in _PROGRAM_CACHE:
        _PROGRAM_CACHE[S] = _build_program(S)
    return _PROGRAM_CACHE[S]


def _make_mask():
    pp = np.arange(_P)[:, None, None]
    tt = np.arange(4)[None, :, None]
    cc = np.arange(512)[None, None, :]
    return ((128 * tt + pp) <= cc).astype(np.float32)


def _round_fp32r(a):
    """Round float32 to the fp32r grid (11-bit mantissa, low 12 bits zero)."""
    bits = np.ascontiguousarray(a, np.float32).view(np.uint32)
    r = ((bits.astype(np.uint64) + 0x800) & 0xFFFFF000).astype(np.uint32)
    return r.view(np.float32)


def make_in_maps(x, w_qkv, w_out):
    x = np.ascontiguousarray(np.asarray(x, np.float32))
    w_qkv = np.ascontiguousarray(np.asarray(w_qkv, np.float32))
    w_out = np.ascontiguousarray(np.asarray(w_out, np.float32))
    E = _E
    mask = _make_mask()
    xTs = [_round_fp32r(x[b].T) for b in range(_B)]
    wqs, wos = [], []
    for g in range(4):
        W = np.concatenate(
            [
                w_qkv[256 * g:256 * (g + 1)],
                w_qkv[E + 256 * g:E + 256 * (g + 1)],
                w_qkv[2 * E + 256 * g:2 * E + 256 * (g + 1)],
            ],
            axis=0,
        )  # [768, E]
        wqs.append(_round_fp32r(W.T))                        # [E, 768]
        wos.append(_round_fp32r(w_out[:, 256 * g:256 * (g + 1)].T))  # [256, E]
    in_maps = []
    for core in range(8):
        b, g = core // 4, core % 4
        in_maps.append(
            {"xT": xTs[b], "wqkvT": wqs[g], "woutT": wos[g], "mask": mask}
        )
    return in_maps


LAST_TRACE_DIR = None


def _enable_jax_compile_cache():
    try:
        import jax

        jax.config.update("jax_compilation_cache_dir", "/tmp/jax_cache")
        jax.config.update("jax_persistent_cache_min_compile_time_secs", 0.0)
        jax.config.update("jax_persistent_cache_min_entry_size_bytes", -1)
    except Exception:
        pass


def kernel(x, w_qkv, w_out, b_out):
    global LAST_EXEC_TIME_NS, LAST_TRACE_DIR
    from concourse.bass_utils import run_bass_kernel_spmd

    _enable_jax_compile_cache()
    b_out = np.asarray(b_out, np.float32)
    in_maps = make_in_maps(x, w_qkv, w_out)
    nc = _get_program()
    trace = bool(int(os.environ.get("BASS_PROFILE", "0")))
    tmpdir = None
    if trace:
        import tempfile

        tmpdir = tempfile.mkdtemp(prefix="bass_trace_")
        LAST_TRACE_DIR = tmpdir
    res = run_bass_kernel_spmd(
        nc, in_maps, core_ids=list(range(8)), trace=trace, tmpdir=tmpdir
    )
    LAST_EXEC_TIME_NS = res.exec_time_ns
    out = np.zeros((_B, _S, _E), np.float32)
    for core in range(8):
        out[core // 4] += res.results[core]["out"]
    out += b_out[None, None, :]
    return out


# revision 25
# speedup vs baseline: 1.2391x; 1.0265x over previous
"""Causal self-attention kernel for 8 TRN2 NeuronCores.

Sharding: core = b*4 + g  (b = batch 0..1, g = head-group 0..3, 4 heads each).
Each core computes, for its batch b and its 4 heads:
  qkv projection -> per-head causal attention (softmax without max-subtraction,
  scores are bounded ~N(0,1)) -> partial output projection over its 256
  attn columns.  Host sums the 4 per-batch partials and adds the bias.

On-device layout (per core):
  xT     [E=1024, S=2048]  host-pretransposed x[b].T  (contraction dim on partitions)
  wqkvT  [E, F=768]        host-built [Wq_g; Wk_g; Wv_g].T
  woutT  [256, E]          host-built w_out[:, 256g:256g+256].T
  mask   [128, 4, 512]     causal mask tiles for the diagonal superblocks
  out    [S, E]            partial output (pre-bias)

All matmuls run as float32r (full-rate fp32 on the PE array).
"""

import os

import numpy as np

_B, _S, _E = 2, 2048, 1024
_H, _D = 16, 64
_F = 768  # per-core qkv rows: 4 heads * 3 * 64
_P = 128

# stash of the last profiled exec time (ns), for test harnesses
LAST_EXEC_TIME_NS = None

_PROGRAM_CACHE = {}


def _build_program(S=_S):
    import concourse.bacc as bacc
    import concourse.mybir as mybir
    import concourse.tile as tile

    f32 = mybir.dt.float32
    f32r = mybir.dt.float32r
    cdt = f32r  # compute dtype for all matmul operands
    Exp = mybir.ActivationFunctionType.Exp

    P = _P
    E, F = _E, _F
    NCH = E // P          # 8 contraction chunks for the projections
    NSB = S // 512        # s-blocks of 512
    NIB = S // 512        # i-blocks (attention query blocks)

    nc = bacc.Bacc("TRN2", target_bir_lowering=False, debug=False)

    xT = nc.declare_dram_parameter("xT", [E, S], cdt, isOutput=False)
    wqkvT = nc.declare_dram_parameter("wqkvT", [E, F], cdt, isOutput=False)
    woutT = nc.declare_dram_parameter("woutT", [256, E], cdt, isOutput=False)
    maskd = nc.declare_dram_parameter("mask", [P, 4, 512], cdt, isOutput=False)
    outd = nc.declare_dram_parameter("out", [S, E], f32, isOutput=True)

    x3 = xT[:].rearrange("(ko p) s -> p ko s", p=P)      # [128, 8, S]
    w3 = wqkvT[:].rearrange("(ko p) f -> p ko f", p=P)   # [128, 8, 768]
    wo3 = woutT[:].rearrange("(c p) e -> p c e", p=P)    # [128, 2, 1024]

    with tile.TileContext(nc) as tc:
        with (
            tc.tile_pool(name="consts", bufs=1) as consts,
            tc.tile_pool(name="xpool", bufs=2) as xpool,
            tc.tile_pool(name="qkpool", bufs=1) as qkpool,
            tc.tile_pool(name="vpool", bufs=1) as vpool,
            tc.tile_pool(name="atpool", bufs=1) as atpool,
            tc.tile_pool(name="probs", bufs=3) as probs,
            tc.tile_pool(name="small", bufs=4) as small,
            tc.tile_pool(name="outpool", bufs=3) as outpool,
            tc.tile_pool(name="psum", bufs=2, space="PSUM") as psum,
        ):
            # ---- constants ----
            w_sb = consts.tile([P, NCH, F], cdt)
            for ch in range(NCH):
                nc.sync.dma_start(w_sb[:, ch], w3[:, ch])
            wo_sb = consts.tile([P, 2, E], cdt)
            mask_sb = consts.tile([P, 4, 512], cdt)

            # per-s-block persistent activations (split tiles so the Tile
            # scheduler can start attention as soon as its block is ready)
            qk_t = [qkpool.tile([P, 4, 512], cdt, name=f"qk{s}") for s in range(NSB)]
            v_t = [vpool.tile([P, 4, 4 * 65], cdt, name=f"v{s}") for s in range(NSB)]
            at_t = [atpool.tile([P, 2, 512], cdt, name=f"at{s}") for s in range(NIB)]
            v4 = [v_t[s].rearrange("p t (h e) -> p t h e", h=4) for s in range(NSB)]

            # ones column of v_aug (row-sum trick for softmax denominators);
            # mask[:, 3, 511] is all-ones, broadcast-DMA it into the gaps
            for s in range(NSB):
                nc.sync.dma_start(
                    v_t[s].rearrange("p t e -> p (t e)")[:, 64::65],
                    maskd[:, 3, 511:512].to_broadcast((P, 4 * 4)),
                )

            def emit_proj(sbk):
                """qkv projection for s-block sbk (PE-dense filler work)."""
                s0 = 512 * sbk
                xt = xpool.tile([P, NCH, 512], cdt, tag="xt", name=f"xt{sbk}")
                for ch in range(NCH):
                    nc.sync.dma_start(xt[:, ch], x3[:, ch, s0:s0 + 512])
                # qT / kT : out rows = W columns (f), weights stationary
                for ft in range(4):
                    qkps = psum.tile([P, 512], f32, tag="acc", bufs=4,
                                     name=f"qkps{sbk}_{ft}")
                    for ch in range(NCH):
                        nc.tensor.matmul(
                            qkps[:],
                            lhsT=w_sb[:, ch, 128 * ft:128 * (ft + 1)],
                            rhs=xt[:, ch, :],
                            start=(ch == 0), stop=(ch == NCH - 1),
                        )
                    nc.vector.tensor_copy(qk_t[sbk][:, ft, :], qkps[:])
                # v : out rows = s, x stationary
                for st in range(4):
                    vps = psum.tile([P, 256], f32, tag="acc", bufs=4,
                                    name=f"vps{sbk}_{st}")
                    for ch in range(NCH):
                        nc.tensor.matmul(
                            vps[:],
                            lhsT=xt[:, ch, 128 * st:128 * (st + 1)],
                            rhs=w_sb[:, ch, 512:768],
                            start=(ch == 0), stop=(ch == NCH - 1),
                        )
                    nc.scalar.activation(
                        v4[sbk][:, st, :, 0:64],
                        vps.rearrange("p (h e) -> p h e", h=4),
                        mybir.ActivationFunctionType.Copy,
                    )

            emit_proj(0)
            nc.sync.dma_start(mask_sb[:], maskd[:])
            nc.sync.dma_start(wo_sb[:], wo3[:])

            # ---- attention (ib outer), with the next s-block projection and
            # ---- this block's out-projection interleaved to keep PE dense
            for ib in range(NIB):
                i0 = 512 * ib
                rs_ib = small.tile([97, 512], f32, tag="rs", name=f"rs{ib}")
                nc.vector.memset(rs_ib[:], 1.0)
                for h in range(4):
                    po = 64 * (h % 2)
                    koq = h // 2
                    kok = 2 + h // 2
                    atps = psum.tile([65, 512], f32, tag="acc", bufs=4,
                                     name=f"atps{h}_{ib}")
                    nsb = 2 * (ib + 1)  # superblocks of 2 j-tiles (causal)
                    for sb2 in range(nsb):
                        d0 = sb2 == 2 * ib      # diagonal superblock (t=0,1)
                        d1 = sb2 == 2 * ib + 1  # diagonal superblock (t=2,3)
                        # fully-masked columns skipped on the d1 superblock
                        e0 = 256 if d1 else 0
                        scps = psum.tile([P, 2, 512], f32, tag="sc", bufs=2,
                                         name=f"scps{h}_{ib}_{sb2}")
                        pb = probs.tile([P, 2, 512], cdt, tag="pb",
                                        name=f"pb{h}_{ib}_{sb2}")
                        for jj in range(2):
                            jt = 2 * sb2 + jj
                            lt = jt % 4
                            nc.tensor.matmul(
                                scps[:, jj, e0:],
                                lhsT=qk_t[jt // 4][po:po + 64, kok,
                                                  128 * lt:128 * (lt + 1)],
                                rhs=qk_t[ib][po:po + 64, koq, e0:],
                                start=True, stop=True, skip_group_check=True,
                            )
                        # probs = exp(scores / sqrt(D)); no max-subtraction
                        nc.scalar.activation(pb[:, :, e0:], scps[:, :, e0:],
                                             Exp, scale=0.125)
                        for jj in range(2):
                            jt = 2 * sb2 + jj
                            t = jt - 4 * ib
                            if d0 or d1:
                                # causal mask; extended strip covers the
                                # fully-masked + partial columns
                                m0, m1 = (0, 128 * t + 128) if d0 else (256, 512)
                                nc.vector.tensor_mul(
                                    pb[:, jj, m0:m1], pb[:, jj, m0:m1],
                                    mask_sb[:, t, m0:m1],
                                )
                            c0 = min(128 * t, 256) if (d0 or d1) and t > 0 else 0
                            nc.tensor.matmul(
                                atps[:, c0:],
                                lhsT=v4[jt // 4][:, jt % 4, h, :],
                                rhs=pb[:, jj, c0:],
                                start=(sb2 == 0 and jj == 0),
                                stop=(sb2 == nsb - 1 and jj == 1),
                                skip_group_check=True,
                            )
                    # stash rowsum + unnormalized attnT; free the psum early
                    nc.scalar.activation(rs_ib[32 * h:32 * h + 1, :], atps[64:65, :],
                                         mybir.ActivationFunctionType.Copy)
                    nc.vector.tensor_copy(
                        at_t[ib][po:po + 64, koq, :], atps[0:64, :]
                    )
                # batched softmax denominators for the 4 heads of this i-block
                rs_inv = small.tile([97, 512], cdt, tag="rsi", name=f"rsi{ib}")
                with nc.allow_low_precision(reason="fp32r softmax denominators"):
                    nc.vector.reciprocal(rs_inv[:], rs_ib[:])
                rs3 = small.tile([1, 512], cdt, tag="rs3", name=f"rs3{ib}")
                nc.scalar.activation(rs3[:], rs_inv[96:97, :],
                                     mybir.ActivationFunctionType.Copy)
                for h in range(4):
                    po = 64 * (h % 2)
                    koq = h // 2
                    bcps = psum.tile([64, 512], f32, tag="acc", bufs=4,
                                     name=f"bcps{h}_{ib}")
                    # mask[p, 0, c>=128] == 1 for every p: free all-ones rows
                    rhs_rec = rs3[:] if h == 3 else rs_inv[32 * h:32 * h + 1, :]
                    _o = 0 if h == 3 else 32 * h
                    lhs_ones = mask_sb[_o:_o + 1, 0, 128:192]
                    nc.tensor.matmul(
                        bcps[:], lhsT=lhs_ones,
                        rhs=rhs_rec,
                        start=True, stop=True, skip_group_check=True,
                    )
                    nc.vector.tensor_mul(
                        at_t[ib][po:po + 64, koq, :],
                        at_t[ib][po:po + 64, koq, :], bcps[:]
                    )
                # next s-block projection: PE filler during exp stalls
                if ib + 1 < NSB:
                    emit_proj(ib + 1)
                # out-proj for this i-block
                for its in range(4):
                    it = 4 * ib + its
                    ot = outpool.tile([P, E], f32, tag="ot", name=f"ot{it}")
                    for ec in range(2):
                        ops = psum.tile([P, 512], f32, tag="acc", bufs=4,
                                        name=f"ops{it}_{ec}")
                        for c in range(2):
                            nc.tensor.matmul(
                                ops[:],
                                lhsT=at_t[ib][:, c, 128 * its:128 * (its + 1)],
                                rhs=wo_sb[:, c, 512 * ec:512 * (ec + 1)],
                                start=(c == 0), stop=(c == 1),
                                skip_group_check=True,
                            )
                        nc.vector.tensor_copy(ot[:, 512 * ec:512 * (ec + 1)], ops[:])
                    nc.sync.dma_start(outd[128 * it:128 * (it + 1), :], ot[:])

    nc.compile()
    return nc


def _get_program(S=_S):
    if S not in _PROGRAM_CACHE:
        _PROGRAM_CACHE[S] = _build_program(S)
    return _PROGRAM_CACHE[S]


def _make_mask():
    pp = np.arange(_P)[:, None, None]
    tt = np.arange(4)[None, :, None]
    cc = np.arange(512)[None, None, :]
    return ((128 * tt + pp) <= cc).astype(np.float32)


def _round_fp32r(a):
    """Round float32 to the fp32r grid (11-bit mantissa, low 12 bits zero)."""
    bits = np.ascontiguousarray(a, np.float32).view(np.uint32)
    r = ((bits.astype(np.uint64) + 0x800) & 0xFFFFF000).astype(np.uint32)
    return r.view(np.float32)


def make_in_maps(x, w_qkv, w_out):
    x = np.ascontiguousarray(np.asarray(x, np.float32))
    w_qkv = np.ascontiguousarray(np.asarray(w_qkv, np.float32))
    w_out = np.ascontiguousarray(np.asarray(w_out, np.float32))
    E = _E
    mask = _make_mask()
    xTs = [_round_fp32r(x[b].T) for b in range(_B)]
    wqs, wos = [], []
    for g in range(4):
        W = np.concatenate(
            [
                w_qkv[256 * g:256 * (g + 1)],
                w_qkv[E + 256 * g:E + 256 * (g + 1)],
                w_qkv[2 * E + 256 * g:2 * E + 256 * (g + 1)],
            ],
            axis=0,
        )  # [768, E]
        wqs.append(_round_fp32r(W.T))                        # [E, 768]
        wos.append(_round_fp32r(w_out[:, 256 * g:256 * (g + 1)].T))  # [256, E]
    in_maps = []
    for core in range(8):
        b, g = core // 4, core % 4
        in_maps.append(
            {"xT": xTs[b], "wqkvT": wqs[g], "woutT": wos[g], "mask": mask}
        )
    return in_maps


LAST_TRACE_DIR = None


def _enable_jax_compile_cache():
    try:
        import jax

        jax.config.update("jax_compilation_cache_dir", "/tmp/jax_cache")
        jax.config.update("jax_persistent_cache_min_compile_time_secs", 0.0)
        jax.config.update("jax_persistent_cache_min_entry_size_bytes", -1)
    except Exception:
        pass


def kernel(x, w_qkv, w_out, b_out):
    global LAST_EXEC_TIME_NS, LAST_TRACE_DIR
    from concourse.bass_utils import run_bass_kernel_spmd

    _enable_jax_compile_cache()
    b_out = np.asarray(b_out, np.float32)
    in_maps = make_in_maps(x, w_qkv, w_out)
    nc = _get_program()
    trace = bool(int(os.environ.get("BASS_PROFILE", "0")))
    tmpdir = None
    if trace:
        import tempfile

        tmpdir = tempfile.mkdtemp(prefix="bass_trace_")
        LAST_TRACE_DIR = tmpdir
    res = run_bass_kernel_spmd(
        nc, in_maps, core_ids=list(range(8)), trace=trace, tmpdir=tmpdir
    )
    LAST_EXEC_TIME_NS = res.exec_time_ns
    out = np.zeros((_B, _S, _E), np.float32)
    for core in range(8):
        out[core // 4] += res.results[core]["out"]
    out += b_out[None, None, :]
    return out


# revision 26
# speedup vs baseline: 1.2608x; 1.0176x over previous
"""Causal self-attention kernel for 8 TRN2 NeuronCores.

Sharding: core = b*4 + g  (b = batch 0..1, g = head-group 0..3, 4 heads each).
Each core computes, for its batch b and its 4 heads:
  qkv projection -> per-head causal attention (softmax without max-subtraction,
  scores are bounded ~N(0,1)) -> partial output projection over its 256
  attn columns.  Host sums the 4 per-batch partials and adds the bias.

On-device layout (per core):
  xT     [E=1024, S=2048]  host-pretransposed x[b].T  (contraction dim on partitions)
  wqkvT  [E, F=768]        host-built [Wq_g; Wk_g; Wv_g].T
  woutT  [256, E]          host-built w_out[:, 256g:256g+256].T
  mask   [128, 4, 512]     causal mask tiles for the diagonal superblocks
  out    [S, E]            partial output (pre-bias)

All matmuls run as float32r (full-rate fp32 on the PE array).
"""

import os

import numpy as np

_B, _S, _E = 2, 2048, 1024
_H, _D = 16, 64
_F = 768  # per-core qkv rows: 4 heads * 3 * 64
_P = 128

# stash of the last profiled exec time (ns), for test harnesses
LAST_EXEC_TIME_NS = None

_PROGRAM_CACHE = {}


def _build_program(S=_S):
    import concourse.bacc as bacc
    import concourse.mybir as mybir
    import concourse.tile as tile

    f32 = mybir.dt.float32
    f32r = mybir.dt.float32r
    cdt = f32r  # compute dtype for all matmul operands
    Exp = mybir.ActivationFunctionType.Exp

    P = _P
    E, F = _E, _F
    NCH = E // P          # 8 contraction chunks for the projections
    NSB = S // 512        # s-blocks of 512
    NIB = S // 512        # i-blocks (attention query blocks)

    nc = bacc.Bacc("TRN2", target_bir_lowering=False, debug=False)

    xT = nc.declare_dram_parameter("xT", [E, S], cdt, isOutput=False)
    wqkvT = nc.declare_dram_parameter("wqkvT", [E, F], cdt, isOutput=False)
    woutT = nc.declare_dram_parameter("woutT", [256, E], cdt, isOutput=False)
    maskd = nc.declare_dram_parameter("mask", [P, 4, 512], cdt, isOutput=False)
    outd = nc.declare_dram_parameter("out", [S, E], f32, isOutput=True)

    x3 = xT[:].rearrange("(ko p) s -> p ko s", p=P)      # [128, 8, S]
    w3 = wqkvT[:].rearrange("(ko p) f -> p ko f", p=P)   # [128, 8, 768]
    wo3 = woutT[:].rearrange("(c p) e -> p c e", p=P)    # [128, 2, 1024]

    with tile.TileContext(nc) as tc:
        with (
            tc.tile_pool(name="consts", bufs=1) as consts,
            tc.tile_pool(name="xpool", bufs=2) as xpool,
            tc.tile_pool(name="qkpool", bufs=1) as qkpool,
            tc.tile_pool(name="vpool", bufs=1) as vpool,
            tc.tile_pool(name="atpool", bufs=1) as atpool,
            tc.tile_pool(name="probs", bufs=3) as probs,
            tc.tile_pool(name="small", bufs=4) as small,
            tc.tile_pool(name="outpool", bufs=3) as outpool,
            tc.tile_pool(name="psum", bufs=2, space="PSUM") as psum,
        ):
            # ---- constants ----
            w_sb = consts.tile([P, NCH, F], cdt)
            wo_sb = consts.tile([P, 2, E], cdt)
            mask_sb = consts.tile([P, 4, 512], cdt)

            # per-s-block persistent activations (split tiles so the Tile
            # scheduler can start attention as soon as its block is ready)
            qk_t = [qkpool.tile([P, 4, 512], cdt, name=f"qk{s}") for s in range(NSB)]
            v_t = [vpool.tile([P, 4, 4 * 65], cdt, name=f"v{s}") for s in range(NSB)]
            at_t = [atpool.tile([P, 2, 512], cdt, name=f"at{s}") for s in range(NIB)]
            v4 = [v_t[s].rearrange("p t (h e) -> p t h e", h=4) for s in range(NSB)]

            # ones column of v_aug (row-sum trick for softmax denominators);
            # mask[:, 3, 511] is all-ones, broadcast-DMA it into the gaps
            for s in range(NSB):
                nc.sync.dma_start(
                    v_t[s].rearrange("p t e -> p (t e)")[:, 64::65],
                    maskd[:, 3, 511:512].to_broadcast((P, 4 * 4)),
                )

            def emit_proj(sbk):
                """qkv projection for s-block sbk (PE-dense filler work)."""
                s0 = 512 * sbk
                xt = xpool.tile([P, NCH, 512], cdt, tag="xt", name=f"xt{sbk}")
                for ch in range(NCH):
                    nc.gpsimd.dma_start(xt[:, ch], x3[:, ch, s0:s0 + 512])
                    if sbk == 0:
                        nc.sync.dma_start(w_sb[:, ch], w3[:, ch])
                # qT / kT : out rows = W columns (f), weights stationary
                for ft in range(4):
                    qkps = psum.tile([P, 512], f32, tag="acc", bufs=4,
                                     name=f"qkps{sbk}_{ft}")
                    for ch in range(NCH):
                        nc.tensor.matmul(
                            qkps[:],
                            lhsT=w_sb[:, ch, 128 * ft:128 * (ft + 1)],
                            rhs=xt[:, ch, :],
                            start=(ch == 0), stop=(ch == NCH - 1),
                        )
                    nc.vector.tensor_copy(qk_t[sbk][:, ft, :], qkps[:])
                # v : out rows = s, x stationary
                for st in range(4):
                    vps = psum.tile([P, 256], f32, tag="acc", bufs=4,
                                    name=f"vps{sbk}_{st}")
                    for ch in range(NCH):
                        nc.tensor.matmul(
                            vps[:],
                            lhsT=xt[:, ch, 128 * st:128 * (st + 1)],
                            rhs=w_sb[:, ch, 512:768],
                            start=(ch == 0), stop=(ch == NCH - 1),
                        )
                    nc.scalar.activation(
                        v4[sbk][:, st, :, 0:64],
                        vps.rearrange("p (h e) -> p h e", h=4),
                        mybir.ActivationFunctionType.Copy,
                    )

            emit_proj(0)
            nc.sync.dma_start(mask_sb[:], maskd[:])
            nc.sync.dma_start(wo_sb[:], wo3[:])

            def _delayed_outproj(ib):
                out = []
                if ib > 0:
                    out += [(its, 4 * (ib - 1) + its) for its in range(4)]
                if ib == NIB - 1:
                    out += [(its, 4 * ib + its) for its in range(4)]
                return out

            # ---- attention (ib outer), with the next s-block projection and
            # ---- the previous block's out-projection interleaved
            for ib in range(NIB):
                i0 = 512 * ib
                rs_ib = small.tile([97, 512], f32, tag="rs", name=f"rs{ib}")
                nc.vector.memset(rs_ib[:], 1.0)
                for h in range(4):
                    po = 64 * (h % 2)
                    koq = h // 2
                    kok = 2 + h // 2
                    atps = psum.tile([65, 512], f32, tag="acc", bufs=4,
                                     name=f"atps{h}_{ib}")
                    nsb = 2 * (ib + 1)  # superblocks of 2 j-tiles (causal)
                    for sb2 in range(nsb):
                        d0 = sb2 == 2 * ib      # diagonal superblock (t=0,1)
                        d1 = sb2 == 2 * ib + 1  # diagonal superblock (t=2,3)
                        # fully-masked columns skipped on the d1 superblock
                        e0 = 256 if d1 else 0
                        scps = psum.tile([P, 2, 512], f32, tag="sc", bufs=2,
                                         name=f"scps{h}_{ib}_{sb2}")
                        pb = probs.tile([P, 2, 512], cdt, tag="pb",
                                        name=f"pb{h}_{ib}_{sb2}")
                        for jj in range(2):
                            jt = 2 * sb2 + jj
                            lt = jt % 4
                            nc.tensor.matmul(
                                scps[:, jj, e0:],
                                lhsT=qk_t[jt // 4][po:po + 64, kok,
                                                  128 * lt:128 * (lt + 1)],
                                rhs=qk_t[ib][po:po + 64, koq, e0:],
                                start=True, stop=True, skip_group_check=True,
                            )
                        # probs = exp(scores / sqrt(D)); no max-subtraction
                        nc.scalar.activation(pb[:, :, e0:], scps[:, :, e0:],
                                             Exp, scale=0.125)
                        for jj in range(2):
                            jt = 2 * sb2 + jj
                            t = jt - 4 * ib
                            if d0 or d1:
                                # causal mask; extended strip covers the
                                # fully-masked + partial columns
                                m0, m1 = (0, 128 * t + 128) if d0 else (256, 512)
                                nc.vector.tensor_mul(
                                    pb[:, jj, m0:m1], pb[:, jj, m0:m1],
                                    mask_sb[:, t, m0:m1],
                                )
                            c0 = min(128 * t, 256) if (d0 or d1) and t > 0 else 0
                            nc.tensor.matmul(
                                atps[:, c0:],
                                lhsT=v4[jt // 4][:, jt % 4, h, :],
                                rhs=pb[:, jj, c0:],
                                start=(sb2 == 0 and jj == 0),
                                stop=(sb2 == nsb - 1 and jj == 1),
                                skip_group_check=True,
                            )
                    # stash rowsum + unnormalized attnT; free the psum early
                    nc.scalar.activation(rs_ib[32 * h:32 * h + 1, :], atps[64:65, :],
                                         mybir.ActivationFunctionType.Copy)
                    nc.vector.tensor_copy(
                        at_t[ib][po:po + 64, koq, :], atps[0:64, :]
                    )
                # batched softmax denominators for the 4 heads of this i-block
                rs_inv = small.tile([97, 512], cdt, tag="rsi", name=f"rsi{ib}")
                with nc.allow_low_precision(reason="fp32r softmax denominators"):
                    nc.vector.reciprocal(rs_inv[:], rs_ib[:])
                rs3 = small.tile([1, 512], cdt, tag="rs3", name=f"rs3{ib}")
                nc.scalar.activation(rs3[:], rs_inv[96:97, :],
                                     mybir.ActivationFunctionType.Copy)
                for h in range(4):
                    po = 64 * (h % 2)
                    koq = h // 2
                    bcps = psum.tile([64, 512], f32, tag="acc", bufs=4,
                                     name=f"bcps{h}_{ib}")
                    # mask[p, 0, c>=128] == 1 for every p: free all-ones rows
                    rhs_rec = rs3[:] if h == 3 else rs_inv[32 * h:32 * h + 1, :]
                    _o = 0 if h == 3 else 32 * h
                    lhs_ones = mask_sb[_o:_o + 1, 0, 128:192]
                    nc.tensor.matmul(
                        bcps[:], lhsT=lhs_ones,
                        rhs=rhs_rec,
                        start=True, stop=True, skip_group_check=True,
                    )
                    nc.vector.tensor_mul(
                        at_t[ib][po:po + 64, koq, :],
                        at_t[ib][po:po + 64, koq, :], bcps[:]
                    )
                # next s-block projection: PE filler during exp stalls
                if ib + 1 < NSB:
                    emit_proj(ib + 1)
                # out-proj, delayed one i-block so it fills the next block's
                # ACT-bound stretches instead of stalling the boundary
                for its, it in _delayed_outproj(ib):
                    ot = outpool.tile([P, E], f32, tag="ot", name=f"ot{it}")
                    for ec in range(2):
                        ops = psum.tile([P, 512], f32, tag="acc", bufs=4,
                                        name=f"ops{it}_{ec}")
                        for c in range(2):
                            nc.tensor.matmul(
                                ops[:],
                                lhsT=at_t[it // 4][:, c, 128 * its:128 * (its + 1)],
                                rhs=wo_sb[:, c, 512 * ec:512 * (ec + 1)],
                                start=(c == 0), stop=(c == 1),
                                skip_group_check=True,
                            )
                        nc.vector.tensor_copy(ot[:, 512 * ec:512 * (ec + 1)], ops[:])
                    nc.sync.dma_start(outd[128 * it:128 * (it + 1), :], ot[:])

    nc.compile()
    return nc


def _get_program(S=_S):
    if S not in _PROGRAM_CACHE:
        _PROGRAM_CACHE[S] = _build_program(S)
    return _PROGRAM_CACHE[S]


def _make_mask():
    pp = np.arange(_P)[:, None, None]
    tt = np.arange(4)[None, :, None]
    cc = np.arange(512)[None, None, :]
    return ((128 * tt + pp) <= cc).astype(np.float32)


def _round_fp32r(a):
    """Round float32 to the fp32r grid (11-bit mantissa, low 12 bits zero)."""
    bits = np.ascontiguousarray(a, np.float32).view(np.uint32)
    r = ((bits.astype(np.uint64) + 0x800) & 0xFFFFF000).astype(np.uint32)
    return r.view(np.float32)


def make_in_maps(x, w_qkv, w_out):
    x = np.ascontiguousarray(np.asarray(x, np.float32))
    w_qkv = np.ascontiguousarray(np.asarray(w_qkv, np.float32))
    w_out = np.ascontiguousarray(np.asarray(w_out, np.float32))
    E = _E
    mask = _make_mask()
    xTs = [_round_fp32r(x[b].T) for b in range(_B)]
    wqs, wos = [], []
    for g in range(4):
        W = np.concatenate(
            [
                w_qkv[256 * g:256 * (g + 1)],
                w_qkv[E + 256 * g:E + 256 * (g + 1)],
                w_qkv[2 * E + 256 * g:2 * E + 256 * (g + 1)],
            ],
            axis=0,
        )  # [768, E]
        wqs.append(_round_fp32r(W.T))                        # [E, 768]
        wos.append(_round_fp32r(w_out[:, 256 * g:256 * (g + 1)].T))  # [256, E]
    in_maps = []
    for core in range(8):
        b, g = core // 4, core % 4
        in_maps.append(
            {"xT": xTs[b], "wqkvT": wqs[g], "woutT": wos[g], "mask": mask}
        )
    return in_maps


LAST_TRACE_DIR = None


def _enable_jax_compile_cache():
    try:
        import jax

        jax.config.update("jax_compilation_cache_dir", "/tmp/jax_cache")
        jax.config.update("jax_persistent_cache_min_compile_time_secs", 0.0)
        jax.config.update("jax_persistent_cache_min_entry_size_bytes", -1)
    except Exception:
        pass


def kernel(x, w_qkv, w_out, b_out):
    global LAST_EXEC_TIME_NS, LAST_TRACE_DIR
    from concourse.bass_utils import run_bass_kernel_spmd

    _enable_jax_compile_cache()
    b_out = np.asarray(b_out, np.float32)
    in_maps = make_in_maps(x, w_qkv, w_out)
    nc = _get_program()
    trace = bool(int(os.environ.get("BASS_PROFILE", "0")))
    tmpdir = None
    if trace:
        import tempfile

        tmpdir = tempfile.mkdtemp(prefix="bass_trace_")
        LAST_TRACE_DIR = tmpdir
    res = run_bass_kernel_spmd(
        nc, in_maps, core_ids=list(range(8)), trace=trace, tmpdir=tmpdir
    )
    LAST_EXEC_TIME_NS = res.exec_time_ns
    out = np.zeros((_B, _S, _E), np.float32)
    for core in range(8):
        out[core // 4] += res.results[core]["out"]
    out += b_out[None, None, :]
    return out
